# revision 16
# baseline (speedup 1.0000x reference)
"""Trainium2 Bass kernel for segmented linear (performer-style) attention.

Problem: nn_Attention_43550968382196 (sparse_attention).
  N=32768 tokens in 64 contiguous equal segments of 512, d_qk=128, d_v=256,
  m=256 random features.  Per segment:
     phi_q = (exp(Uq - hq - rowmax(Uq)) + eps) / sqrt(m)
     phi_k = (exp(Uk - hk - segmax(Uk)) + eps) / sqrt(m)
     out   = (phi_q @ (phi_k^T V)) / (phi_q . sum(phi_k) + 1e-8)

Device math (equivalent to the reference up to rounding):
  * 1/sqrt(m) cancels in the ratio -> unscaled phi, eps_norm' = 1e-8*m.
  * Both exps run RAW (no bias): exp is monotone, so
    rowmax(exp U) == exp(rowmax U) and the stabilizers become
    multiplicative post-factors.
  * Q side: qp = exp(Uq) * s + eps with s = exp(-hq)/rowmax(exp Uq)
    (one fused tensor_scalar per chunk).
  * K side: exp(-hk) is folded into V on the HOST (V' = exp(-hk)*[V|1|0]),
    so KV = exp(Uk)^T @ V' = (exp(Uk-hk))^T [V|1|0] directly.  The
    segment-max normalizer cancels in the ratio; the phi-eps term needs
    eps*exp(segmax), added as a rank-1 PE-accumulated correction
    KV += (eps*g) * ones (x) Vsum with Vsum precomputed on the host and
    g = allreduce-max(exp Uk) from a GpSimd partition all-reduce.  The
    normalizer eps becomes (1e-8*m)*g, added per row.
  * All big matmuls run in bf16 (host-prerounded inputs; device exps and
    copies produce bf16 operands); PSUM accumulation stays fp32.

DMA strategy: the per-core DMA throughput is limited by descriptor
processing (~280 ns per descriptor, 16 engines; one descriptor per
partition line), NOT by HBM bandwidth.  So every transfer is one big DMA
with maximal per-partition contiguous lines: K^T (+omega +identity packed
as extra columns), Q^T (+exp(-hq)), whole V in device layout (16.5 KB
lines), and the output staged in SBUF and stored in two 4-segment DMAs
(8 KB lines).  ~640 descriptors total vs ~3200 for the naive layout.

Sharding: 64 segments split 8-per-core across 8 NeuronCores (data parallel,
no collectives).  Per core the 8 segments run through a 2-deep software
pipeline: iteration s queues U-matmuls(s), KV+transpose(s-1), output(s-2)
back-to-back on the PE so it never waits on the exp/copy chains.
"""

import math
import os
import sys

for _p in ("/opt/trn_rl_repo",):
    if _p not in sys.path and os.path.isdir(_p):
        sys.path.insert(0, _p)

import numpy as np
import ml_dtypes

import concourse.bass as bass
import concourse.bacc as bacc
import concourse.tile as tile
from concourse import mybir
from concourse import bass_isa
from concourse.bass_utils import run_bass_kernel_spmd

F32 = mybir.dt.float32
BF16 = mybir.dt.bfloat16
AF = mybir.ActivationFunctionType
ALU = mybir.AluOpType
AX = mybir.AxisListType
RED = bass_isa.ReduceOp

N_CORES = 8
N = 32768
D = 128          # qk dim
M = 256          # features
DV = 256         # v dim
DVA = 258        # device V columns: [V | 1 | 0]
P = 128          # partitions / tokens per chunk
NSEG = 8         # segments per core
CH = 4           # chunks per segment
MC = 2           # m chunks (256 / 128)
TOK = NSEG * 512
HSEG = NSEG // 2          # segments per output-store batch

KTX = TOK + M + P         # K^T cols + omega + identity
QTX = TOK + NSEG * CH     # Q^T cols + exp(-hq)

EPS_PHI = 1e-4
EPS_NORM2 = 1e-8 * M
H_SCALE = 1.0 / (2.0 * math.sqrt(D))


def build_nc():
    nc = bacc.Bacc("TRN2", target_bir_lowering=False, debug=False)

    KTd = nc.declare_dram_parameter("KTX", [D, KTX], BF16, isOutput=False)
    QTd = nc.declare_dram_parameter("QTX", [D, QTX], BF16, isOutput=False)
    Vd = nc.declare_dram_parameter("V", [P, NSEG * CH * DVA], BF16,
                                   isOutput=False)
    VSd = nc.declare_dram_parameter("vsum", [1, NSEG * DVA], F32,
                                    isOutput=False)
    Od = nc.declare_dram_parameter("out", [P, NSEG * CH * DV], BF16,
                                   isOutput=True)

    with tile.TileContext(nc) as tc:
        with (
            tc.tile_pool(name="const", bufs=1) as const,
            tc.tile_pool(name="sb", bufs=2) as sb,
            tc.tile_pool(name="sm", bufs=4) as sm,
            tc.tile_pool(name="ps", bufs=1, space="PSUM") as ps,
        ):
            # one descriptor-friendly DMA per tensor (see module docstring),
            # split across both HWDGE rings (SP + Activation) to double the
            # per-ring DMA bandwidth cap
            kT_x = const.tile([D, KTX], BF16, name="kT_x")
            nc.sync.dma_start(kT_x[:, :], KTd[:, :])
            qT_x = const.tile([D, QTX], BF16, name="qT_x")
            nc.scalar.dma_start(qT_x[:, :], QTd[:, :])
            v_all = const.tile([P, NSEG, CH, DVA], BF16, name="v_all")
            vv = Vd[:, :].rearrange("p (s c d) -> p s c d", s=NSEG, c=CH)
            nc.sync.dma_start(v_all[:, 0:HSEG], vv[:, 0:HSEG])
            nc.scalar.dma_start(v_all[:, HSEG:NSEG], vv[:, HSEG:NSEG])
            vsum_all = const.tile([1, NSEG, DVA], F32, name="vsum_all")
            nc.scalar.dma_start(
                vsum_all[:, :, :],
                VSd[:, :].rearrange("p (s d) -> p s d", s=NSEG))
            onesr_t = const.tile([1, P], BF16, name="onesr_t")
            nc.vector.memset(onesr_t[:, :], 1.0)

            kT_all = kT_x[:, 0:TOK]
            omega_t = kT_x[:, TOK:TOK + M]
            ident_r = kT_x[:, TOK + M:TOK + M + P]
            qT_all = qT_x[:, 0:TOK]
            ehq_all = qT_x[:, TOK:QTX].rearrange("p (s c) -> p s c", s=NSEG)

            # output staging: two 4-segment batches, stored as 8KB lines
            ost = [const.tile([P, HSEG, CH, DV], BF16, name=f"ost{h}")
                   for h in range(2)]

            stA = {}
            stB = {}

            def head(s):
                """U matmuls + exps + maxes + eps factors for segment s."""
                uk = ps.tile([P, CH, M], F32, name=f"uk{s}", tag="uk", bufs=1)
                uq = ps.tile([P, CH, M], F32, name=f"uq{s}", tag="uq", bufs=1)
                for c in range(CH):
                    nc.tensor.matmul(uk[:, c, :],
                                     kT_all[:, bass.ts(s * CH + c, P)],
                                     omega_t[:, :])
                for c in range(CH):
                    nc.tensor.matmul(uq[:, c, :],
                                     qT_all[:, bass.ts(s * CH + c, P)],
                                     omega_t[:, :])

                ek = sb.tile([P, CH, M], BF16, name=f"ek{s}", tag="ek",
                             bufs=3)
                nc.scalar.activation(ek[:, :, :], uk[:, :, :], AF.Exp)
                eq = sb.tile([P, CH, M], BF16, name=f"eq{s}", tag="eq",
                             bufs=3)
                nc.scalar.activation(eq[:, :, :], uq[:, :, :], AF.Exp)

                # K segment max -> g = exp(segmax) on every partition
                kmx = sm.tile([P, 1], F32, name=f"kmx{s}", tag="kmx")
                nc.vector.tensor_reduce(kmx[:, :], ek[:, :, :], axis=AX.XY,
                                        op=ALU.max)
                gmax = sm.tile([P, 1], F32, name=f"gmax{s}", tag="gmax")
                nc.gpsimd.partition_all_reduce(gmax[:, 0:1], kmx[:, 0:1],
                                               channels=P, reduce_op=RED.max)
                cen = sm.tile([P, 1], F32, name=f"cen{s}", tag="cen")
                nc.vector.tensor_scalar_mul(cen[:, :], gmax[:, :], EPS_NORM2)
                cvs = sm.tile([1, DVA], BF16, name=f"cvs{s}", tag="cvs")
                nc.vector.tensor_scalar(cvs[:, :], vsum_all[0:1, s, :],
                                        gmax[0:1, 0:1], EPS_PHI,
                                        op0=ALU.mult, op1=ALU.mult)

                # Q per-row scale s = exp(-hq) / rowmax(exp Uq)
                qmx = sm.tile([P, CH], BF16, name=f"qmx{s}", tag="qmx")
                nc.vector.tensor_reduce(qmx[:, :], eq[:, :, :], axis=AX.X,
                                        op=ALU.max)
                rq = sm.tile([P, CH], F32, name=f"rq{s}", tag="rq")
                nc.vector.reciprocal(rq[:, :], qmx[:, :])
                sq = sm.tile([P, CH], F32, name=f"sq{s}", tag="sq")
                nc.vector.tensor_tensor(sq[:, :], ehq_all[:, s, :], rq[:, :],
                                        op=ALU.mult)
                qp = sb.tile([P, CH, M], BF16, name=f"qp{s}", tag="qp",
                             bufs=3)
                for c in range(CH):
                    nc.vector.tensor_scalar(qp[:, c, :], eq[:, c, :],
                                            sq[:, c:c + 1], EPS_PHI,
                                            op0=ALU.mult, op1=ALU.add)
                stA[s] = (ek, qp, cvs, cen)

            def mid(s):
                """KV matmuls + Qp transposes for segment s."""
                ek, qp, cvs, cen = stA.pop(s)
                kv_sb = sb.tile([P, MC, DVA], BF16, name=f"kvsb{s}",
                                tag="kvsb", bufs=3)
                qpT_sb = sb.tile([P, MC, 512], BF16, name=f"qpTsb{s}",
                                 tag="qpTsb", bufs=3)
                for mc in range(MC):
                    kvp = ps.tile([P, 512], F32, name=f"kv{s}_{mc}", tag="W",
                                  bufs=4)
                    for c in range(CH):
                        nc.tensor.matmul(kvp[:, 0:DVA],
                                         ek[:, c, bass.ts(mc, P)],
                                         v_all[:, s, c, :],
                                         start=(c == 0), stop=False)
                    nc.tensor.matmul(kvp[:, 0:DVA], onesr_t[0:1, :],
                                     cvs[0:1, :], start=False, stop=True)
                    nc.vector.tensor_copy(kv_sb[:, mc, :], kvp[:, 0:DVA])
                    qpTp = ps.tile([P, 512], BF16, name=f"qpT{s}_{mc}",
                                   tag="W", bufs=4)
                    for c in range(CH):
                        nc.tensor.transpose(qpTp[:, bass.ts(c, P)],
                                            qp[:, c, bass.ts(mc, P)],
                                            ident_r[:, :])
                    if mc == 0:
                        nc.scalar.copy(qpT_sb[:, mc, :], qpTp[:, :])
                    else:
                        nc.vector.tensor_copy(qpT_sb[:, mc, :], qpTp[:, :])
                stB[s] = (kv_sb, qpT_sb, cen)

            def tail(s):
                """Numerator, normalization and staged output for segment s."""
                kv_sb, qpT_sb, cen = stB.pop(s)
                ot = ost[s // HSEG]
                for c in range(CH):
                    nm = ps.tile([P, 512], F32, name=f"nm{s}_{c}",
                                 tag="W", bufs=4)
                    for mc in range(MC):
                        nc.tensor.matmul(nm[:, 0:DVA],
                                         qpT_sb[:, mc, bass.ts(c, P)],
                                         kv_sb[:, mc, :],
                                         start=(mc == 0),
                                         stop=(mc == MC - 1))
                    den = sm.tile([P, 1], F32, name=f"den{s}_{c}", tag="den")
                    nc.vector.tensor_tensor(den[:, :], nm[:, DV:DV + 1],
                                            cen[:, 0:1], op=ALU.add)
                    rr = sm.tile([P, 1], F32, name=f"rr{s}_{c}", tag="rr")
                    nc.vector.reciprocal(rr[:, :], den[:, :])
                    nc.scalar.activation(ot[:, s % HSEG, c, :], nm[:, 0:DV],
                                         AF.Copy, scale=rr[:, 0:1])
                if s % HSEG == HSEG - 1:
                    h = s // HSEG
                    nc.gpsimd.dma_start(
                        Od[:, bass.ts(h, HSEG * CH * DV)]
                        .rearrange("p (s c d) -> p s c d", s=HSEG, c=CH),
                        ot[:, :, :, :])

            # 2-deep software pipeline (see module docstring)
            for s in range(NSEG):
                head(s)
                if s >= 1:
                    mid(s - 1)
                if s >= 2:
                    tail(s - 2)
            mid(NSEG - 1)
            tail(NSEG - 2)
            tail(NSEG - 1)

    nc.compile()
    return nc


_NC_CACHE = {}


def _get_nc():
    if "nc" not in _NC_CACHE:
        _NC_CACHE["nc"] = build_nc()
    return _NC_CACHE["nc"]


def _bf16(x):
    return np.ascontiguousarray(x.astype(ml_dtypes.bfloat16))


def make_in_maps(Q, K, V, omega):
    Q = np.ascontiguousarray(np.asarray(Q, dtype=np.float32))
    K = np.ascontiguousarray(np.asarray(K, dtype=np.float32))
    hscale = np.float32(H_SCALE)
    ehq = np.exp(-(Q * Q).sum(axis=1) * hscale)          # exp(-hq)  [N]
    ehk = np.exp(-(K * K).sum(axis=1) * hscale)          # exp(-hk)  [N]
    V = np.asarray(V, dtype=np.float32)
    Vaug = np.zeros((V.shape[0], DVA), np.float32)
    Vaug[:, :DV] = V
    Vaug[:, DV] = 1.0
    # per-segment column sums of [V | 1 | 0] (fp32, host-side)
    vsum = Vaug.reshape(N // 512, 512, DVA).sum(axis=1)  # [64, DVA]
    Vp16 = _bf16(Vaug * ehk[:, None])                    # exp(-hk)-folded V
    # device layout: [P, (s c d)] per core
    Vdev = Vp16.reshape(N_CORES, NSEG, CH, P, DVA).transpose(0, 3, 1, 2, 4)
    Vdev = np.ascontiguousarray(
        Vdev.reshape(N_CORES, P, NSEG * CH * DVA))
    omega = np.asarray(omega, dtype=np.float32)
    omega16 = (omega * np.float32(D ** -0.25)).astype(ml_dtypes.bfloat16)
    ident16 = np.eye(P, dtype=ml_dtypes.bfloat16)
    QT16 = _bf16(Q.T)
    KT16 = _bf16(K.T)
    ehq16 = ehq.astype(ml_dtypes.bfloat16)
    in_maps = []
    for c in range(N_CORES):
        sl = slice(c * TOK, (c + 1) * TOK)
        ktx = np.concatenate([KT16[:, sl], omega16, ident16], axis=1)
        ehq_dev = (ehq16[sl].reshape(NSEG, CH, P)
                   .transpose(2, 0, 1).reshape(P, NSEG * CH))
        qtx = np.concatenate([QT16[:, sl], ehq_dev], axis=1)
        in_maps.append({
            "KTX": np.ascontiguousarray(ktx),
            "QTX": np.ascontiguousarray(qtx),
            "V": Vdev[c],
            "vsum": np.ascontiguousarray(
                vsum[c * NSEG:(c + 1) * NSEG].reshape(1, NSEG * DVA)),
        })
    return in_maps


def unpack_out(res):
    # out arrives in device layout [P, (s c d)] bf16 per core
    outs = np.stack([np.asarray(res.results[c]["out"])
                     for c in range(N_CORES)])
    outs = outs.reshape(N_CORES, P, NSEG, CH, DV).transpose(0, 2, 3, 1, 4)
    return np.ascontiguousarray(
        outs.reshape(N, DV).astype(np.float32))


def kernel(Q, K, V, omega, num_batch, batch_seg):
    nc = _get_nc()
    in_maps = make_in_maps(Q, K, V, omega)
    res = run_bass_kernel_spmd(nc, in_maps, core_ids=list(range(N_CORES)))
    return unpack_out(res)


# revision 20
# speedup vs baseline: 1.1932x; 1.1932x over previous
"""Trainium2 Bass kernel for segmented linear (performer-style) attention.

Problem: nn_Attention_43550968382196 (sparse_attention).
  N=32768 tokens in 64 contiguous equal segments of 512, d_qk=128, d_v=256,
  m=256 random features.  Per segment:
     phi_q = (exp(Uq - hq - rowmax(Uq)) + eps) / sqrt(m)
     phi_k = (exp(Uk - hk - segmax(Uk)) + eps) / sqrt(m)
     out   = (phi_q @ (phi_k^T V)) / (phi_q . sum(phi_k) + 1e-8)

Device math (equivalent to the reference up to rounding):
  * 1/sqrt(m) cancels in the ratio -> unscaled phi, eps_norm' = 1e-8*m.
  * Both exps run RAW (no bias): exp is monotone, so
    rowmax(exp U) == exp(rowmax U) and the stabilizers become
    multiplicative post-factors.
  * Q side: qp = exp(Uq) * s + eps with s = exp(-hq)/rowmax(exp Uq)
    (one fused tensor_scalar per chunk).
  * K side: exp(-hk) is folded into V on the HOST (V' = exp(-hk)*[V|1|0]),
    so KV = exp(Uk)^T @ V' = (exp(Uk-hk))^T [V|1|0] directly.  The
    segment-max normalizer cancels in the ratio; the phi-eps term needs
    eps*exp(segmax), added as a rank-1 PE-accumulated correction
    KV += (eps*g) * ones (x) Vsum with Vsum precomputed on the host and
    g = allreduce-max(exp Uk) from a GpSimd partition all-reduce.  The
    normalizer eps becomes (1e-8*m)*g, added per row.
  * All big matmuls run in bf16 (host-prerounded inputs; device exps and
    copies produce bf16 operands); PSUM accumulation stays fp32.

DMA strategy: the per-core DMA throughput is limited by descriptor
processing (~280 ns per descriptor, 16 engines; one descriptor per
partition line), NOT by HBM bandwidth.  So every transfer is one big DMA
with maximal per-partition contiguous lines: K^T (+omega +identity packed
as extra columns), Q^T (+exp(-hq)), whole V in device layout (16.5 KB
lines), and the output staged in SBUF and stored in two 4-segment DMAs
(8 KB lines).  ~640 descriptors total vs ~3200 for the naive layout.

Sharding: 64 segments split 8-per-core across 8 NeuronCores (data parallel,
no collectives).  Per core the 8 segments run through a 2-deep software
pipeline: iteration s queues U-matmuls(s), KV+transpose(s-1), output(s-2)
back-to-back on the PE so it never waits on the exp/copy chains.
"""

import math
import os
import sys

for _p in ("/opt/trn_rl_repo",):
    if _p not in sys.path and os.path.isdir(_p):
        sys.path.insert(0, _p)

import numpy as np
import ml_dtypes

import concourse.bass as bass
import concourse.bacc as bacc
import concourse.tile as tile
from concourse import mybir
from concourse import bass_isa
from concourse.bass_utils import run_bass_kernel_spmd

F32 = mybir.dt.float32
BF16 = mybir.dt.bfloat16
AF = mybir.ActivationFunctionType
ALU = mybir.AluOpType
AX = mybir.AxisListType
RED = bass_isa.ReduceOp

N_CORES = 8
N = 32768
D = 128          # qk dim
M = 256          # features
DV = 256         # v dim
DVA = 258        # device V columns: [V | 1 | 0]
P = 128          # partitions / tokens per chunk
NSEG = 8         # segments per core
CH = 4           # chunks per segment
MC = 2           # m chunks (256 / 128)
TOK = NSEG * 512
HSEG = NSEG // 2          # segments per V-load half
OSEG = 2                  # segments per output-store batch

KTX = TOK + M + P         # K^T cols + omega + identity
QTX = TOK + NSEG * CH     # Q^T cols + exp(-hq)

EPS_PHI = 1e-4
EPS_NORM2 = 1e-8 * M
H_SCALE = 1.0 / (2.0 * math.sqrt(D))


def build_nc():
    nc = bacc.Bacc("TRN2", target_bir_lowering=False, debug=False)

    KTd = nc.declare_dram_parameter("KTX", [D, KTX], BF16, isOutput=False)
    QTd = nc.declare_dram_parameter("QTX", [D, QTX], BF16, isOutput=False)
    Vd = nc.declare_dram_parameter("V", [P, NSEG * CH * DVA], BF16,
                                   isOutput=False)
    VSd = nc.declare_dram_parameter("vsum", [1, NSEG * DVA], F32,
                                    isOutput=False)
    Od = nc.declare_dram_parameter("out", [P, NSEG * CH * DV], BF16,
                                   isOutput=True)

    with tile.TileContext(nc) as tc:
        with (
            tc.tile_pool(name="const", bufs=1) as const,
            tc.tile_pool(name="sb", bufs=2) as sb,
            tc.tile_pool(name="sm", bufs=4) as sm,
            tc.tile_pool(name="ps", bufs=1, space="PSUM") as ps,
        ):
            # one descriptor-friendly DMA per tensor (see module docstring),
            # split across both HWDGE rings (SP + Activation) to double the
            # per-ring DMA bandwidth cap
            kT_x = const.tile([D, KTX], BF16, name="kT_x")
            nc.sync.dma_start(kT_x[:, :], KTd[:, :])
            qT_x = const.tile([D, QTX], BF16, name="qT_x")
            nc.scalar.dma_start(qT_x[:, :], QTd[:, :])
            v_all = const.tile([P, NSEG, CH, DVA], BF16, name="v_all")
            vv = Vd[:, :].rearrange("p (s c d) -> p s c d", s=NSEG, c=CH)
            nc.sync.dma_start(v_all[:, 0:HSEG], vv[:, 0:HSEG])
            nc.scalar.dma_start(v_all[:, HSEG:NSEG], vv[:, HSEG:NSEG])
            vsum_all = const.tile([1, NSEG, DVA], F32, name="vsum_all")
            nc.scalar.dma_start(
                vsum_all[:, :, :],
                VSd[:, :].rearrange("p (s d) -> p s d", s=NSEG))
            onesr_t = const.tile([1, P], BF16, name="onesr_t")
            nc.vector.memset(onesr_t[:, :], 1.0)

            kT_all = kT_x[:, 0:TOK]
            omega_t = kT_x[:, TOK:TOK + M]
            ident_r = kT_x[:, TOK + M:TOK + M + P]
            qT_all = qT_x[:, 0:TOK]
            ehq_all = qT_x[:, TOK:QTX].rearrange("p (s c) -> p s c", s=NSEG)

            # output staging: 2-segment batches, stored as 4KB lines on the
            # SP ring (free once the inputs have streamed)
            ost = [const.tile([P, OSEG, CH, DV], BF16, name=f"ost{h}")
                   for h in range(NSEG // OSEG)]

            stA = {}
            stB = {}

            def head(s):
                """U matmuls + exps + maxes + eps factors for segment s."""
                uk = ps.tile([P, CH, M], F32, name=f"uk{s}", tag="uk", bufs=1)
                uq = ps.tile([P, CH, M], F32, name=f"uq{s}", tag="uq", bufs=1)
                for c in range(CH):
                    nc.tensor.matmul(uk[:, c, :],
                                     kT_all[:, bass.ts(s * CH + c, P)],
                                     omega_t[:, :])
                for c in range(CH):
                    nc.tensor.matmul(uq[:, c, :],
                                     qT_all[:, bass.ts(s * CH + c, P)],
                                     omega_t[:, :])

                ek = sb.tile([P, CH, M], BF16, name=f"ek{s}", tag="ek",
                             bufs=3)
                nc.scalar.activation(ek[:, :, :], uk[:, :, :], AF.Exp)
                eq = sb.tile([P, CH, M], BF16, name=f"eq{s}", tag="eq",
                             bufs=3)
                nc.scalar.activation(eq[:, :, :], uq[:, :, :], AF.Exp)

                # K segment max -> g = exp(segmax) on every partition
                kmx = sm.tile([P, 1], F32, name=f"kmx{s}", tag="kmx")
                nc.vector.tensor_reduce(kmx[:, :], ek[:, :, :], axis=AX.XY,
                                        op=ALU.max)
                gmax = sm.tile([P, 1], F32, name=f"gmax{s}", tag="gmax")
                nc.gpsimd.partition_all_reduce(gmax[:, 0:1], kmx[:, 0:1],
                                               channels=P, reduce_op=RED.max)
                cen = sm.tile([P, 1], F32, name=f"cen{s}", tag="cen")
                nc.vector.tensor_scalar_mul(cen[:, :], gmax[:, :], EPS_NORM2)
                cvs = sm.tile([1, DVA], BF16, name=f"cvs{s}", tag="cvs")
                nc.vector.tensor_scalar(cvs[:, :], vsum_all[0:1, s, :],
                                        gmax[0:1, 0:1], EPS_PHI,
                                        op0=ALU.mult, op1=ALU.mult)

                # Q per-row scale s = exp(-hq) / rowmax(exp Uq)
                qmx = sm.tile([P, CH], BF16, name=f"qmx{s}", tag="qmx")
                nc.vector.tensor_reduce(qmx[:, :], eq[:, :, :], axis=AX.X,
                                        op=ALU.max)
                rq = sm.tile([P, CH], F32, name=f"rq{s}", tag="rq")
                nc.vector.reciprocal(rq[:, :], qmx[:, :])
                sq = sm.tile([P, CH], F32, name=f"sq{s}", tag="sq")
                nc.vector.tensor_tensor(sq[:, :], ehq_all[:, s, :], rq[:, :],
                                        op=ALU.mult)
                qp = sb.tile([P, CH, M], BF16, name=f"qp{s}", tag="qp",
                             bufs=3)
                for c in range(CH):
                    nc.vector.tensor_scalar(qp[:, c, :], eq[:, c, :],
                                            sq[:, c:c + 1], EPS_PHI,
                                            op0=ALU.mult, op1=ALU.add)
                stA[s] = (ek, qp, cvs, cen)

            def mid(s):
                """KV matmuls + Qp transposes for segment s."""
                ek, qp, cvs, cen = stA.pop(s)
                kv_sb = sb.tile([P, MC, DVA], BF16, name=f"kvsb{s}",
                                tag="kvsb", bufs=3)
                qpT_sb = sb.tile([P, MC, 512], BF16, name=f"qpTsb{s}",
                                 tag="qpTsb", bufs=3)
                for mc in range(MC):
                    kvp = ps.tile([P, 512], F32, name=f"kv{s}_{mc}", tag="W",
                                  bufs=4)
                    for c in range(CH):
                        nc.tensor.matmul(kvp[:, 0:DVA],
                                         ek[:, c, bass.ts(mc, P)],
                                         v_all[:, s, c, :],
                                         start=(c == 0), stop=False)
                    nc.tensor.matmul(kvp[:, 0:DVA], onesr_t[0:1, :],
                                     cvs[0:1, :], start=False, stop=True)
                    nc.vector.tensor_copy(kv_sb[:, mc, :], kvp[:, 0:DVA])
                    qpTp = ps.tile([P, 512], BF16, name=f"qpT{s}_{mc}",
                                   tag="W", bufs=4)
                    for c in range(CH):
                        nc.tensor.transpose(qpTp[:, bass.ts(c, P)],
                                            qp[:, c, bass.ts(mc, P)],
                                            ident_r[:, :])
                    if mc == 0:
                        nc.scalar.copy(qpT_sb[:, mc, :], qpTp[:, :])
                    else:
                        nc.vector.tensor_copy(qpT_sb[:, mc, :], qpTp[:, :])
                stB[s] = (kv_sb, qpT_sb, cen)

            def tail(s):
                """Numerator, normalization and staged output for segment s."""
                kv_sb, qpT_sb, cen = stB.pop(s)
                ot = ost[s // OSEG]
                for c in range(CH):
                    nm = ps.tile([P, 512], F32, name=f"nm{s}_{c}",
                                 tag="W", bufs=4)
                    for mc in range(MC):
                        nc.tensor.matmul(nm[:, 0:DVA],
                                         qpT_sb[:, mc, bass.ts(c, P)],
                                         kv_sb[:, mc, :],
                                         start=(mc == 0),
                                         stop=(mc == MC - 1))
                    den = sm.tile([P, 1], F32, name=f"den{s}_{c}", tag="den")
                    nc.vector.tensor_tensor(den[:, :], nm[:, DV:DV + 1],
                                            cen[:, 0:1], op=ALU.add)
                    rr = sm.tile([P, 1], F32, name=f"rr{s}_{c}", tag="rr")
                    nc.vector.reciprocal(rr[:, :], den[:, :])
                    nc.scalar.activation(ot[:, s % OSEG, c, :], nm[:, 0:DV],
                                         AF.Copy, scale=rr[:, 0:1])
                if s % OSEG == OSEG - 1:
                    h = s // OSEG
                    nc.sync.dma_start(
                        Od[:, bass.ts(h, OSEG * CH * DV)]
                        .rearrange("p (s c d) -> p s c d", s=OSEG, c=CH),
                        ot[:, :, :, :])

            # 2-deep software pipeline (see module docstring)
            for s in range(NSEG):
                head(s)
                if s >= 1:
                    mid(s - 1)
                if s >= 2:
                    tail(s - 2)
            mid(NSEG - 1)
            tail(NSEG - 2)
            tail(NSEG - 1)

    nc.compile()
    return nc


_NC_CACHE = {}


def _get_nc():
    if "nc" not in _NC_CACHE:
        _NC_CACHE["nc"] = build_nc()
    return _NC_CACHE["nc"]


def _bf16(x):
    return np.ascontiguousarray(x.astype(ml_dtypes.bfloat16))


def make_in_maps(Q, K, V, omega):
    Q = np.ascontiguousarray(np.asarray(Q, dtype=np.float32))
    K = np.ascontiguousarray(np.asarray(K, dtype=np.float32))
    hscale = np.float32(H_SCALE)
    ehq = np.exp(-(Q * Q).sum(axis=1) * hscale)          # exp(-hq)  [N]
    ehk = np.exp(-(K * K).sum(axis=1) * hscale)          # exp(-hk)  [N]
    V = np.asarray(V, dtype=np.float32)
    Vaug = np.zeros((V.shape[0], DVA), np.float32)
    Vaug[:, :DV] = V
    Vaug[:, DV] = 1.0
    # per-segment column sums of [V | 1 | 0] (fp32, host-side)
    vsum = Vaug.reshape(N // 512, 512, DVA).sum(axis=1)  # [64, DVA]
    Vp16 = _bf16(Vaug * ehk[:, None])                    # exp(-hk)-folded V
    # device layout: [P, (s c d)] per core
    Vdev = Vp16.reshape(N_CORES, NSEG, CH, P, DVA).transpose(0, 3, 1, 2, 4)
    Vdev = np.ascontiguousarray(
        Vdev.reshape(N_CORES, P, NSEG * CH * DVA))
    omega = np.asarray(omega, dtype=np.float32)
    omega16 = (omega * np.float32(D ** -0.25)).astype(ml_dtypes.bfloat16)
    ident16 = np.eye(P, dtype=ml_dtypes.bfloat16)
    QT16 = _bf16(Q.T)
    KT16 = _bf16(K.T)
    ehq16 = ehq.astype(ml_dtypes.bfloat16)
    in_maps = []
    for c in range(N_CORES):
        sl = slice(c * TOK, (c + 1) * TOK)
        ktx = np.concatenate([KT16[:, sl], omega16, ident16], axis=1)
        ehq_dev = (ehq16[sl].reshape(NSEG, CH, P)
                   .transpose(2, 0, 1).reshape(P, NSEG * CH))
        qtx = np.concatenate([QT16[:, sl], ehq_dev], axis=1)
        in_maps.append({
            "KTX": np.ascontiguousarray(ktx),
            "QTX": np.ascontiguousarray(qtx),
            "V": Vdev[c],
            "vsum": np.ascontiguousarray(
                vsum[c * NSEG:(c + 1) * NSEG].reshape(1, NSEG * DVA)),
        })
    return in_maps


def unpack_out(res):
    # out arrives in device layout [P, (s c d)] bf16 per core
    outs = np.stack([np.asarray(res.results[c]["out"])
                     for c in range(N_CORES)])
    outs = outs.reshape(N_CORES, P, NSEG, CH, DV).transpose(0, 2, 3, 1, 4)
    return np.ascontiguousarray(
        outs.reshape(N, DV).astype(np.float32))


def kernel(Q, K, V, omega, num_batch, batch_seg):
    nc = _get_nc()
    in_maps = make_in_maps(Q, K, V, omega)
    res = run_bass_kernel_spmd(nc, in_maps, core_ids=list(range(N_CORES)))
    return unpack_out(res)
